# revision 24
# baseline (speedup 1.0000x reference)
"""Trainium2 Bass kernel for a transformer decoder layer (self-attn + cross-attn + FFN).

Sharding: 2-way data-parallel over batch x 4-way sequence-parallel over tokens.
Core i handles batch b = i//4, and within the batch group the 16 query tiles
(128 rows each) are dealt round-robin: core r gets global tiles {r, 4+r, 8+r,
12+r} in local order L=0..3.  With a causal tgt_mask this balances the
skippable score work: local tile L only needs key tiles 0..4L+3 (ceil 4L+4),
identical on every core, so the single SPMD program skips 37.5% of the SA
score/exp/AV work.  K/V are computed on the token shard and AllGathered
within the 4-core group; global key order is recovered by indexing the
gathered buffer at (r = t%4, lt = t//4).

On-device layout is feature-major: activations live as x^T[d, s].  Matmul
inputs are bf16; the attention core (scores, exp, AV) runs in fp8e4 with
DoubleRow perf mode: scores contract K8 against a (Q_hi, Q_residual) pair
(compensating Q's fp8 quantization), and AV contracts two key tiles per
instruction.  The softmax denominator comes from ones columns appended to V
(cols 64 and 129 of a 130-wide layout; even head uses cols 0:65, odd head
65:130, so both AV outputs sit at partitions 0..64).  exp() is computed with
bias -2 so fp8e4's 448 ceiling is safe; softmax is shift-invariant.  The
residual stream (pre-LN sums, LN inputs/outputs used as residuals) stays in
f32; only matmul operands are rounded to bf16/fp8.
"""

import math

import numpy as np
import ml_dtypes

import concourse.bass as bass
import concourse.bacc as bacc
import concourse.mybir as mybir
import concourse.tile as tile
from concourse.bass_utils import run_bass_kernel_spmd

B, S, D, H, DK, DFF = 2, 2048, 1024, 16, 64, 4096
LN_EPS = 1e-5
N_CORES = 8
GROUP = 4                     # cores per batch group
T = S // GROUP                # 512 token rows per core
NLT = T // 128                # 4 local query tiles per core
NDT = D // 128                # 8 feature tiles
NKT = S // 128                # 16 key tiles
NFT = DFF // 128              # 32 ffn tiles
FFN_SPLIT = 4                 # ffn dff passes (SBUF pressure)
REPLICA_GROUPS = [[0, 1, 2, 3], [4, 5, 6, 7]]

F32 = mybir.dt.float32
F32R = mybir.dt.float32r
BF16 = mybir.dt.bfloat16
FP8 = mybir.dt.float8e4
U8 = mybir.dt.uint8
AF = mybir.ActivationFunctionType
OP = mybir.AluOpType
DR = mybir.MatmulPerfMode.DoubleRow
MD = BF16            # dtype of bf16 matmul-feeding SBUF tiles
NP_MD = ml_dtypes.bfloat16
EXP_BIAS = -2.0      # exp(s*scale + bias): shift-invariant, keeps fp8 in range

# vecs row indices (packed host-side into one [13, D] input)
V_SABQ, V_SABK, V_CABQ, V_CABK, V_SABO, V_CABO, V_FFB2, \
    V_LN1G, V_LN1B, V_LN2G, V_LN2B, V_LN3G, V_LN3B = range(13)

# sa_mode / ca_mode: 0 = no mask (all-ones), 1 = causal-skip windows,
# 2 = general mask on every key tile
SKIP_CEILS = [4 * (L + 1) for L in range(NLT)]   # kt tiles per local q tile
FULL_CEILS = [NKT] * NLT

_KERNELS: dict[tuple[int, int], bass.Bass] = {}
LAST_VARIANT = (0, 0)


def _f32(ap):
    return ap.bitcast(F32)


def _build(sa_mode: int, ca_mode: int, stub_collectives: bool = False) -> bass.Bass:
    """stub_collectives=True replaces AllGathers with local DMA copies so the
    module can run under single-core TimelineSim (timing analysis only)."""
    nc = bacc.Bacc("TRN2", target_bir_lowering=False,
                   num_devices=1 if stub_collectives else N_CORES)

    xT = nc.dram_tensor("xT", [D, T], MD, kind="ExternalInput")
    xF = nc.dram_tensor("xF", [D, T], F32, kind="ExternalInput")
    encT = nc.dram_tensor("encT", [D, T], MD, kind="ExternalInput")
    w_in = {}
    for p in ("sa", "ca"):
        for n in ("q", "k", "v", "o"):
            # host-retiled: [out-chunk, p, j, o]
            w_in[f"{p}_w{n}"] = nc.dram_tensor(f"{p}_w{n}", [NDT, 128, NDT, 128],
                                               MD, kind="ExternalInput")
    ff_w1 = nc.dram_tensor("ff_w1", [NFT, 128, NDT, 128], MD, kind="ExternalInput")
    ff_w2 = nc.dram_tensor("ff_w2", [NDT, 128, NFT, 128], MD, kind="ExternalInput")
    vecs = nc.dram_tensor("vecs", [13, D], F32, kind="ExternalInput")
    ffb1 = nc.dram_tensor("ffb1", [DFF], F32, kind="ExternalInput")
    masks = {}
    for p, mode in (("sa", sa_mode), ("ca", ca_mode)):
        if mode == 1:
            # per local q tile: last two key-tile pairs of its range
            masks[p] = nc.dram_tensor(f"{p}_maskw", [NLT, 2, 2, 128, 128], U8,
                                      kind="ExternalInput")
        elif mode == 2:
            masks[p] = nc.dram_tensor(f"{p}_maskf", [NKT // 2, 2, 128, T], U8,
                                      kind="ExternalInput")
    outT = nc.dram_tensor("outT", [D, T], F32, kind="ExternalOutput")

    from contextlib import ExitStack
    with tile.TileContext(nc) as tc, ExitStack() as ctx:
        _emit(ctx, nc, tc, xT, xF, encT, w_in, ff_w1, ff_w2, vecs, ffb1, masks,
              outT, sa_mode, ca_mode, stub_collectives)
    nc.compile()
    return nc


def _emit(ctx, nc, tc, xT, xF, encT, w_in, ff_w1, ff_w2, vecs, ffb1, masks,
          outT, sa_mode, ca_mode, stub_collectives=False):
    ex = ctx.enter_context
    fp = ex(tc.tile_pool(name="persist", bufs=1))
    wp = ex(tc.tile_pool(name="weights", bufs=2))
    sp = ex(tc.tile_pool(name="work", bufs=2))
    pp = ex(tc.tile_pool(name="psum", bufs=2, space="PSUM"))
    dram = ex(tc.tile_pool(name="dram", bufs=1, space="DRAM"))

    # ---- persistent activations first: xT feeds the very first matmuls ----
    xT_sb = fp.tile([128, NDT, T], MD, tag="slotA", name="xT_sb")
    nc.sync.dma_start(xT_sb[:], xT.ap().rearrange("(j p) s -> p j s", p=128))
    xF_sb = fp.tile([128, NDT, T], F32, tag="slotF", name="xF_sb")
    nc.sync.dma_start(xF_sb[:], xF.ap().rearrange("(j p) s -> p j s", p=128))

    # ---- constants / small params ----
    vec_sb = fp.tile([128, 13, NDT], F32, name="vec_sb")
    nc.sync.dma_start(vec_sb[:], vecs.ap().rearrange("v (j p) -> p v j", p=128))
    ffb1_sb = fp.tile([128, NFT], F32, name="ffb1_sb")
    ones32_sb = fp.tile([128, 32], F32, name="ones32_sb")
    nc.vector.memset(ones32_sb[:], 1.0)
    ones_r = fp.tile([128, 1], F32R, name="ones_r")
    nc.vector.tensor_copy(ones_r[:], ones32_sb[:, 0:1])
    eps_sb = fp.tile([1, 1], F32, name="eps_sb")
    nc.vector.memset(eps_sb[:], LN_EPS)
    expb_sb = fp.tile([128, 1], F32, name="expb_sb")
    nc.vector.memset(expb_sb[:], EXP_BIAS)

    def vcol(i, j):
        return vec_sb[:, i, j:j + 1]

    encT_sb = fp.tile([128, NDT, T], MD, tag="slotB", name="encT_sb")

    def w_chunk(name, dt):
        """[128, NDT, 128] chunk dt of a retiled weight."""
        c = wp.tile([128, NDT, 128], MD, tag="w", name=f"{name}_c{dt}")
        nc.sync.dma_start(c[:], w_in[name].ap()[dt])
        return c

    # ================= K/V shard projections + AllGather =================
    kv_full = {}

    def make_kv(pre, src_sb):
        if pre == "ca":
            nc.sync.dma_start(
                encT_sb[:], encT.ap().rearrange("(j p) s -> p j s", p=128))
        bk_i = V_SABK if pre == "sa" else V_CABK
        kT_sh = dram.tile([D, T], MD, name=f"{pre}_kT_sh")
        for dt in range(NDT):
            wc = w_chunk(f"{pre}_wk", dt)
            ps = pp.tile([128, T], F32, tag="mm", name="kv_ps")
            for j in range(NDT):
                nc.tensor.matmul(ps[:], wc[:, j, :], src_sb[:, j, :],
                                 start=(j == 0), stop=(j == NDT - 1))
            kt_sb = sp.tile([128, T], MD, tag="k8stage", name="kt_sb")
            nc.vector.tensor_scalar_add(kt_sb[:], ps[:], vcol(bk_i, dt))
            nc.sync.dma_start(kT_sh[dt * 128:(dt + 1) * 128, :], kt_sb[:])

        # V layout: [pair, hh, s, 128]: per head cols [V(64) | ones | zeros];
        # DoubleRow lhsT needs contiguous [2, 128] rows and M in {64, 128},
        # so the ones/denominator column rides in a padded 128-wide row.
        v_sh = dram.tile([H // 2, 2, T, 128], FP8, name=f"{pre}_v_sh")
        for vt in range(D // 512):
            wv = wp.tile([128, 4, NDT, 128], MD, tag="wv", name=f"{pre}_wv{vt}")
            nc.sync.dma_start(
                wv[:], w_in[f"{pre}_wv"].ap()[4 * vt:4 * vt + 4]
                .rearrange("d p j o -> p d j o"))
            for st in range(T // 128):
                ps = pp.tile([128, 512], F32, tag="mm", name="v_ps")
                for j in range(NDT):
                    nc.tensor.matmul(ps[:],
                                     src_sb[:, j, st * 128:(st + 1) * 128],
                                     wv[:, :, j, :],
                                     start=(j == 0), stop=(j == NDT - 1))
                v_sb = sp.tile([128, 4, 2, 128], FP8, tag="v8stage", name="v_sb")
                psv = ps[:].rearrange("p (pl hh c) -> p pl hh c", pl=4, hh=2)
                nc.vector.tensor_copy(v_sb[:, :, :, 0:64], psv)
                nc.vector.memset(v_sb[:, :, :, 64:65], 1.0)
                nc.vector.memset(v_sb[:, :, :, 65:128], 0.0)
                nc.sync.dma_start(
                    v_sh[vt * 4:(vt + 1) * 4, :, st * 128:(st + 1) * 128, :]
                    .rearrange("pl hh s c -> s pl hh c"), v_sb[:])

        kT_full = dram.tile([GROUP * D, T], MD, name=f"{pre}_kT_full")
        v_full = dram.tile([GROUP * (H // 2), 2, T, 128], FP8,
                           name=f"{pre}_v_full")
        if stub_collectives:
            for r in range(GROUP):
                nc.sync.dma_start(kT_full[r * D:(r + 1) * D, :], kT_sh[:])
                nc.sync.dma_start(
                    v_full[r * (H // 2):(r + 1) * (H // 2), :, :, :], v_sh[:])
        else:
            nc.gpsimd.collective_compute("AllGather", OP.bypass,
                                         ins=[kT_sh.opt()], outs=[kT_full.opt()],
                                         replica_groups=REPLICA_GROUPS)
            nc.gpsimd.collective_compute("AllGather", OP.bypass,
                                         ins=[v_sh.opt()], outs=[v_full.opt()],
                                         replica_groups=REPLICA_GROUPS)
        kv_full[pre] = (kT_full, v_full)

    make_kv("sa", xT_sb)

    # ================= LN =================
    def layer_norm(pre_sb, g_i, b_i, emit_out):
        """Per-token LN of feature-major f32 pre_sb [128, NDT, T]."""
        ps_sum = pp.tile([1, T], F32, tag="av_ps", name="ln_sum")
        ps_sq = pp.tile([1, T], F32, tag="av_ps", name="ln_sq")
        for j in range(NDT):
            nc.tensor.matmul(ps_sum[:], ones_r[:], pre_sb[:, j, :],
                             start=(j == 0), stop=(j == NDT - 1))
        for j in range(NDT):
            sq = sp.tile([128, T], F32R, tag="stage", name="ln_sq_t")
            nc.vector.tensor_tensor(sq[:], _f32(pre_sb[:, j, :]),
                                    _f32(pre_sb[:, j, :]), OP.mult)
            nc.tensor.matmul(ps_sq[:], ones_r[:], sq[:],
                             start=(j == 0), stop=(j == NDT - 1))
        mean = sp.tile([1, T], F32, tag="sm1", name="ln_mean")
        nc.vector.tensor_scalar_mul(mean[:], ps_sum[:], 1.0 / D)
        m2 = sp.tile([1, T], F32, tag="sm2", name="ln_m2")
        nc.vector.tensor_tensor(m2[:], mean[:], mean[:], OP.mult)
        var = sp.tile([1, T], F32, tag="sm3", name="ln_var")
        nc.vector.scalar_tensor_tensor(var[:], ps_sq[:], 1.0 / D, m2[:],
                                       OP.mult, OP.subtract)
        std = sp.tile([1, T], F32, tag="sm4", name="ln_std")
        nc.scalar.activation(std[:], var[:], AF.Sqrt, bias=eps_sb[:])
        rstd = sp.tile([1, T], F32, tag="sm5", name="ln_rstd")
        nc.vector.reciprocal(rstd[:], std[:])
        meanB = sp.tile([128, T], F32, tag="bc1", name="ln_meanB")
        nc.gpsimd.partition_broadcast(meanB[:], mean[:])
        rstdB = sp.tile([128, T], F32, tag="bc2", name="ln_rstdB")
        nc.gpsimd.partition_broadcast(rstdB[:], rstd[:])
        for j in range(NDT):
            t1 = sp.tile([128, T], F32, tag="stage", name="ln_t1")
            nc.vector.scalar_tensor_tensor(t1[:], _f32(pre_sb[:, j, :]), 0.0,
                                           meanB[:], OP.bypass, OP.subtract)
            t2 = sp.tile([128, T], F32, tag="stage2", name="ln_t2")
            nc.vector.scalar_tensor_tensor(t2[:], t1[:], vcol(g_i, j), rstdB[:],
                                           OP.mult, OP.mult)
            emit_out(j, t2, vcol(b_i, j))

    def ln_into(dst_bf, dst_f32):
        def emit(j, t2, bias):
            nc.vector.tensor_scalar_add(dst_bf[:, j, :], t2[:], bias)
            nc.vector.tensor_scalar_add(dst_f32[:, j, :], t2[:], bias)
        return emit

    # ================= attention =================
    x1T_sb = fp.tile([128, NDT, T], MD, tag="slotD", name="x1T_sb")
    x1F_sb = fp.tile([128, NDT, T], F32, tag="slotG", name="x1F_sb")
    x2T_sb = fp.tile([128, NDT, T], MD, tag="slotA", name="x2T_sb")
    x2F_sb = fp.tile([128, NDT, T], F32, tag="slotF", name="x2F_sb")

    def attention(pre, mode, qsrc_sb, bq_i, bo_i, residF_sb, g_i, b_i,
                  out_bf, out_f32, kvp, post_core=None):
        kT_full, v_full = kv_full[pre]
        ceils = SKIP_CEILS if mode == 1 else FULL_CEILS
        pairs = [c // 2 for c in ceils]          # kt pairs per local q tile
        npair = max(pairs)
        scale = 1.0 / math.sqrt(DK)

        qT_sb = fp.tile([128, NDT, T], MD, tag="slotC", name=f"{pre}_qT")
        for dt in range(NDT):
            wc = w_chunk(f"{pre}_wq", dt)
            ps = pp.tile([128, T], F32, tag="mm", name="q_ps")
            for j in range(NDT):
                nc.tensor.matmul(ps[:], wc[:, j, :], qsrc_sb[:, j, :],
                                 start=(j == 0), stop=(j == NDT - 1))
            nc.vector.tensor_scalar_add(qT_sb[:, dt, :], ps[:], vcol(bq_i, dt))

        if post_core is not None:
            post_core()
        aoT_sb = fp.tile([128, NDT, T], MD, tag="slotB", name=f"{pre}_aoT")
        ao2_sb = fp.tile([64, NDT, T], MD, tag="aostage", name=f"{pre}_ao2")

        mask_sb = None
        if mode == 1:
            mask_sb = kvp.tile([128, NLT, 2, 2, 128], U8, tag="mask",
                               name=f"{pre}_mask", bufs=1)
            for L in range(NLT):
                for w in range(2):
                    nc.sync.dma_start(
                        mask_sb[:, L, w, :, :],
                        masks[pre].ap()[L, w].rearrange("k p q -> p k q"))
        elif mode == 2:
            mask_sb = kvp.tile([128, NKT // 2, 2, T], U8, tag="mask",
                               name=f"{pre}_mask", bufs=1)
            for w in range(NKT // 2):
                nc.sync.dma_start(
                    mask_sb[:, w, :, :],
                    masks[pre].ap()[w].rearrange("k p q -> p k q"))

        # active-suffix start column for pair index p8
        def s0(p8):
            return 128 * sum(1 for c in pairs if c <= p8)

        for h2 in range(H // 2):            # head pairs
            kh2 = kvp.tile([128, GROUP, T], MD, tag="kh2", name="kh2")
            nc.sync.dma_start(
                kh2[:],
                kT_full[:].rearrange("(r f) s -> f r s", r=GROUP)
                [h2 * 128:(h2 + 1) * 128, :, :])
            vaug = kvp.tile([128, NKT // 2, 2, 2, 128], FP8, tag="vaug",
                            name="vaug")
            vv = vaug[:].rearrange("p pr hh sl c -> p sl hh pr c")
            for r in range(GROUP):
                for hh in range(2):
                    nc.sync.dma_start(
                        vv[:, r % 2, hh, r // 2::2, :],
                        v_full[r * (H // 2) + h2, hh, :, :]
                        .rearrange("(lt p) c -> p lt c", p=128))

            for hh in range(2):
                hb = 64 * hh
                q_sl = qT_sb[hb:hb + 64, h2, :]
                ps_av = pp.tile([128, T], F32, tag="av_ps", name="av_ps")
                for p8 in range(npair):
                    st = s0(p8)
                    act = T - st
                    ps_s = pp.tile([128, 2, 512], F32, tag="sc_ps",
                                   name="score_ps")
                    for i in range(2):
                        t = 2 * p8 + i
                        r, lt = t % GROUP, t // GROUP
                        nc.tensor.matmul(ps_s[:, i, st:],
                                         kh2[hb:hb + 64, r,
                                             lt * 128:(lt + 1) * 128],
                                         q_sl[:, st:], start=True, stop=True)
                    exp8 = sp.tile([128, 2, T], FP8, tag="exp", name="exp8",
                                   bufs=3)
                    nc.scalar.activation(exp8[:, :, st:], ps_s[:, :, st:],
                                         AF.Exp, scale=scale, bias=expb_sb[:])
                    if mode == 1:
                        Lw = p8 // 2
                        nc.vector.tensor_tensor(
                            exp8[:, :, Lw * 128:(Lw + 1) * 128],
                            exp8[:, :, Lw * 128:(Lw + 1) * 128],
                            mask_sb[:, Lw, p8 % 2, :, :], OP.mult)
                    elif mode == 2:
                        nc.vector.tensor_tensor(
                            exp8[:, :, :], exp8[:, :, :],
                            mask_sb[:, p8, :, :], OP.mult)
                    nc.tensor.matmul(ps_av[:, st:],
                                     vaug[:, p8, hh, :, :],
                                     exp8[:, :, st:],
                                     start=(p8 == 0), stop=(p8 == npair - 1),
                                     perf_mode=DR, skip_group_check=True)
                recip = sp.tile([1, T], F32, tag="sm1", name="recip_sb")
                nc.vector.reciprocal(recip[:], ps_av[64:65, :])
                rb = sp.tile([64, T], F32, tag="bc1", name="recip_bc")
                nc.gpsimd.partition_broadcast(rb[:], recip[:])
                if hh == 0:
                    nc.vector.tensor_tensor(aoT_sb[0:64, h2, :], ps_av[0:64, :],
                                            rb[:], OP.mult)
                else:
                    nc.vector.tensor_tensor(ao2_sb[:, h2, :], ps_av[0:64, :],
                                            rb[:], OP.mult)
        # odd heads: partitions 0..64 -> 64..128 in one batched DMA bounce
        nc.sync.dma_start(aoT_sb[64:128, :, :], ao2_sb[:])

        # out-projection + residual (f32) + LN
        pre_ln = fp.tile([128, NDT, T], F32R, tag="slotE", name=f"{pre}_preln")
        for dt in range(NDT):
            wc = w_chunk(f"{pre}_wo", dt)
            ps = pp.tile([128, T], F32, tag="mm", name="o_ps")
            for j in range(NDT):
                nc.tensor.matmul(ps[:], wc[:, j, :], aoT_sb[:, j, :],
                                 start=(j == 0), stop=(j == NDT - 1))
            nc.vector.scalar_tensor_tensor(pre_ln[:, dt, :], ps[:], vcol(bo_i, dt),
                                           residF_sb[:, dt, :], OP.add, OP.add)
        layer_norm(pre_ln, g_i, b_i, ln_into(out_bf, out_f32))

    with tc.tile_pool(name="kv", bufs=2) as kvp:
        attention("sa", sa_mode, xT_sb, V_SABQ, V_SABO, xF_sb, V_LN1G, V_LN1B,
                  x1T_sb, x1F_sb, kvp, post_core=lambda: make_kv("ca", encT_sb))
        attention("ca", ca_mode, x1T_sb, V_CABQ, V_CABO, x1F_sb, V_LN2G, V_LN2B,
                  x2T_sb, x2F_sb, kvp)

    # ================= FFN =================
    ff_preln = fp.tile([128, NDT, T], F32R, tag="slotE", name="ff_preln")
    NSP = NFT // FFN_SPLIT
    nc.sync.dma_start(ffb1_sb[:], ffb1.ap().rearrange("(j p) -> p j", p=128))
    wfp = ex(tc.tile_pool(name="ffnw", bufs=4))
    for half in range(FFN_SPLIT):
        hT_sb = fp.tile([128, NSP, T], MD, tag="slotC", name=f"hT{half}")
        for fi in range(NSP):
            ft = half * NSP + fi
            w1c = wfp.tile([128, NDT, 128], MD, tag="w1c", name="w1c")
            nc.sync.dma_start(w1c[:], ff_w1.ap()[ft])
            ps = pp.tile([128, T], F32, tag="mm", name="h_ps")
            for j in range(NDT):
                nc.tensor.matmul(ps[:], w1c[:, j, :], x2T_sb[:, j, :],
                                 start=(j == 0), stop=(j == NDT - 1))
            nc.vector.tensor_scalar(hT_sb[:, fi, :], ps[:],
                                    ffb1_sb[:, ft:ft + 1], 0.0,
                                    OP.add, OP.max)
        for dt in range(NDT):
            w2c = wfp.tile([128, NSP, 128], MD, tag="w2c", name="w2c")
            nc.sync.dma_start(
                w2c[:], ff_w2.ap()[dt][:, half * NSP:(half + 1) * NSP, :])
            ps = pp.tile([128, T], F32, tag="mm", name="y_ps")
            for fi in range(NSP):
                nc.tensor.matmul(ps[:], w2c[:, fi, :], hT_sb[:, fi, :],
                                 start=(fi == 0), stop=(fi == NSP - 1))
            if half == 0:
                nc.vector.scalar_tensor_tensor(ff_preln[:, dt, :], ps[:],
                                               vcol(V_FFB2, dt),
                                               x2F_sb[:, dt, :], OP.add, OP.add)
            else:
                nc.vector.tensor_tensor(ff_preln[:, dt, :], ps[:],
                                        _f32(ff_preln[:, dt, :]), OP.add)

    def emit_final(j, t2, bias):
        o = sp.tile([128, T], F32, tag="stage2", name="out_t")
        nc.vector.tensor_scalar_add(o[:], t2[:], bias)
        nc.sync.dma_start(outT[j * 128:(j + 1) * 128, :], o[:])

    layer_norm(ff_preln, V_LN3G, V_LN3B, emit_final)


def _get_kernel(sa_mode: int, ca_mode: int) -> bass.Bass:
    key = (sa_mode, ca_mode)
    if key not in _KERNELS:
        _KERNELS[key] = _build(*key)
    return _KERNELS[key]


def _retile(w: np.ndarray, n_out: int) -> np.ndarray:
    """[K, O] f32 -> [O//128, 128(p of K), K//128, 128(o)] in bf16."""
    K, O = w.shape
    nj = K // 128
    r = w.reshape(nj, 128, n_out, 128)          # [j, p, dt, o]
    r = r.transpose(2, 1, 0, 3)                 # [dt, p, j, o]
    return np.ascontiguousarray(r.astype(NP_MD))


def _rows_for(r: int) -> np.ndarray:
    """Local token order for lane r: global 128-row tiles 4L + r."""
    tiles = [4 * L + r for L in range(NLT)]
    return np.concatenate([np.arange(128) + 128 * t for t in tiles])


def _mask_mode(mask: np.ndarray) -> int:
    """0 = all ones; 1 = admissible for causal-style skipping; 2 = general."""
    if np.all(mask != 0):
        return 0
    # admissible iff for every global q tile g, keys beyond tile 4*(g//4)+3
    # are fully masked out
    m = mask.reshape(B, NKT, 128, NKT, 128).any(axis=(2, 4))  # [B, qt, kt]
    for g in range(NKT):
        ceil = 4 * (g // 4) + 4
        if m[:, g, ceil:].any():
            return 2
    return 1


def kernel(**inputs) -> np.ndarray:
    global LAST_VARIANT
    x = np.asarray(inputs["x"], np.float32)
    enc = np.asarray(inputs["enc_output"], np.float32)
    tgt_mask = np.asarray(inputs["tgt_mask"])
    mem_mask = np.asarray(inputs["memory_mask"])
    sa_mode = _mask_mode(tgt_mask)
    ca_mode = _mask_mode(mem_mask)
    LAST_VARIANT = (sa_mode, ca_mode)

    nc = _get_kernel(sa_mode, ca_mode)

    vecs = [np.asarray(inputs[k], np.float32)
            for k in ("sa_bq", "sa_bk", "ca_bq", "ca_bk")]
    for p in ("sa", "ca"):
        wo = np.asarray(inputs[f"{p}_wo"], np.float32)
        bv = np.asarray(inputs[f"{p}_bv"], np.float32)
        bo = np.asarray(inputs[f"{p}_bo"], np.float32)
        vecs.append(wo.T @ bv + bo)
    vecs.append(np.asarray(inputs["ff_b2"], np.float32))
    for i in (1, 2, 3):
        vecs.append(np.asarray(inputs[f"ln{i}_g"], np.float32))
        vecs.append(np.asarray(inputs[f"ln{i}_b"], np.float32))
    vecs_np = np.ascontiguousarray(np.stack(vecs))          # [13, D]

    shared = {}
    for name in ("sa_wq", "sa_wk", "sa_wv", "sa_wo",
                 "ca_wq", "ca_wk", "ca_wv", "ca_wo"):
        shared[name] = _retile(np.asarray(inputs[name], np.float32), NDT)
    shared["ff_w1"] = _retile(np.asarray(inputs["ff_w1"], np.float32), NFT)
    shared["ff_w2"] = _retile(np.asarray(inputs["ff_w2"], np.float32), NDT)
    shared["vecs"] = vecs_np
    shared["ffb1"] = np.ascontiguousarray(np.asarray(inputs["ff_b1"], np.float32))

    def mask_inputs(pre, mode, mask, b, rows):
        if mode == 0:
            return {}
        mb = (mask[b] != 0).astype(np.uint8)        # [q_global, k_global]
        if mode == 1:
            # [L, w, k2, p, q]: key tile t = 4L + 2w + k2, q = local tile L
            out = np.empty((NLT, 2, 2, 128, 128), np.uint8)
            for L in range(NLT):
                qg = rows[L * 128:(L + 1) * 128]
                for w in range(2):
                    for k2 in range(2):
                        t = 4 * L + 2 * w + k2
                        out[L, w, k2] = mb[np.ix_(qg, np.arange(128) + t * 128)].T
            return {f"{pre}_maskw": np.ascontiguousarray(out)}
        # mode 2: [w(8 key pairs), k2, p, q_local]
        out = np.empty((NKT // 2, 2, 128, T), np.uint8)
        for w in range(NKT // 2):
            for k2 in range(2):
                t = 2 * w + k2
                out[w, k2] = mb[np.ix_(rows, np.arange(128) + t * 128)].T
        return {f"{pre}_maskf": np.ascontiguousarray(out)}

    in_maps = []
    for core in range(N_CORES):
        b, r = divmod(core, GROUP)
        rows = _rows_for(r)
        m = dict(shared)
        xT = x[b, rows].T
        m["xT"] = np.ascontiguousarray(xT.astype(NP_MD))
        m["xF"] = np.ascontiguousarray(xT)
        m["encT"] = np.ascontiguousarray(enc[b, rows].T.astype(NP_MD))
        m.update(mask_inputs("sa", sa_mode, tgt_mask, b, rows))
        m.update(mask_inputs("ca", ca_mode, mem_mask, b, rows))
        in_maps.append(m)

    res = run_bass_kernel_spmd(nc, in_maps, core_ids=list(range(N_CORES)))

    out = np.empty((B, S, D), np.float32)
    for core in range(N_CORES):
        b, r = divmod(core, GROUP)
        out[b, _rows_for(r), :] = res.results[core]["outT"].T
    return out
